# revision 1
# baseline (speedup 1.0000x reference)
"""Trainium2 Bass kernel for nn_Decoder (GRU decoder + vocab projection).

Model (per reference):
    h0  = hn @ fc_w^T + fc_b                      [B,H]
    x   = emb[y]                                  [B,S,E]
    gx  = x @ W_ih^T + b_ih                       [B,S,3H]  (precomputed, biases folded)
    GRU scan over S steps (PyTorch gate order r,z,n):
        r = sigmoid(gxr + h@Wr^T + br_hh)
        z = sigmoid(gxz + h@Wz^T + bz_hh)
        n = tanh(gxn + r * (h@Wn^T + bn_hh))
        h = (1-z)*n + z*h
    out = h_seq @ pred_w^T + pred_b               [B,S,V]

Distribution: GRU scan replicated on all 8 cores (its cost is latency-bound,
independent of batch); pred projection and pred weights vocab-sharded 8 ways;
each core writes its [B*S, V/8] logit shard.

On-chip design notes:
  - strips in the gate psum bank (PE column groups): z@[0:16], r@[32:48], n@[64:80]
  - gate psum preloaded with gx via selector matmuls (M=32 covers sim-visible gap rows)
  - sigmoid(z,r) on ACT over psum[0:48]; n-strip evicted raw
  - 4 PE transposes of a combined [80,128] window -> H-layout (128 partitions, FD=64)
  - gate math in H-layout; state update lands directly transposed (next stationary)
  - state also written bf16 into outT (the pred matmul's stationary)
  - pred matmuls interleave into PE idle slots during the scan
  - all heavy matmuls in float32r (1 cyc/row) or bf16; fp32 runs 4x slower on PE
"""
import os
import numpy as np
import ml_dtypes
from contextlib import ExitStack

import concourse.bass as bass
import concourse.tile as tile
from concourse import bacc, mybir
from concourse import bass_utils
from concourse.masks import make_identity

f32 = mybir.dt.float32
f32r = mybir.dt.float32r
bf16 = mybir.dt.bfloat16
i32 = mybir.dt.int32
SIG = mybir.ActivationFunctionType.Sigmoid
TANH = mybir.ActivationFunctionType.Tanh
MUL = mybir.AluOpType.mult
ADD = mybir.AluOpType.add

V, E, EH, H = 32000, 512, 1024, 512
B, S = 16, 128
NC = 8
VS = V // NC          # 4000 vocab shard per core
NT = (B * S) // 128   # 16 (b,t)-tiles of 128 rows, t-major
G3 = 3 * H            # 1536

_PROG_CACHE = {}


def build_program(debug=False, no_pred=False, serial_gx=False):
    key = ("nc", debug, no_pred, serial_gx)
    if key in _PROG_CACHE:
        return _PROG_CACHE[key]
    nc = bacc.Bacc("TRN2", target_bir_lowering=False, debug=False,
                   enable_asserts=False, num_devices=NC)

    # ---------------- DRAM I/O ----------------
    EMB = nc.dram_tensor("emb", [V, E], f32, kind="ExternalInput").ap()
    YT = nc.dram_tensor("y_tm", [B * S, 1], i32, kind="ExternalInput").ap()
    HNT = nc.dram_tensor("hnT", [128, 8 * 16], f32r, kind="ExternalInput").ap()
    FCWT = nc.dram_tensor("fcwT", [128, 8 * 512], f32r, kind="ExternalInput").ap()
    FCBT = nc.dram_tensor("fcbT", [128, 4], f32, kind="ExternalInput").ap()
    WHHT = nc.dram_tensor("whhT", [128, 4 * G3], bf16, kind="ExternalInput").ap()
    WIHT = nc.dram_tensor("wihT", [128, 4 * G3], bf16, kind="ExternalInput").ap()
    BIASRZ = nc.dram_tensor("bias_rz", [128, 1024], bf16, kind="ExternalInput").ap()
    BIASNT = nc.dram_tensor("bias_nT", [128, 4], f32, kind="ExternalInput").ap()
    BNB = nc.dram_tensor("bnb", [32, 512], bf16, kind="ExternalInput").ap()
    SEL = nc.dram_tensor("sel", [128, 64], bf16, kind="ExternalInput").ap()
    PREDWT = nc.dram_tensor("predwT", [128, 4 * VS], bf16, kind="ExternalInput").ap()
    PREDB = nc.dram_tensor("predb", [128, VS], bf16, kind="ExternalInput").ap()
    OUT = nc.dram_tensor("out", [B * S, VS], f32, kind="ExternalOutput").ap()
    if debug:
        DBG_H0 = nc.dram_tensor("dbg_h0", [128, 64], f32, kind="ExternalOutput").ap()
        DBG_GX = nc.dram_tensor("dbg_gx", [128, 1024], f32, kind="ExternalOutput").ap()
        DBG_GXNT = nc.dram_tensor("dbg_gxnt", [128, 256], f32, kind="ExternalOutput").ap()
        DBG_OUTT = nc.dram_tensor("dbg_outT", [128, 256], f32, kind="ExternalOutput").ap()
        DBG_PG = nc.dram_tensor("dbg_pg", [128, 512], f32, kind="ExternalOutput").ap()
        DBG_GT = nc.dram_tensor("dbg_gt", [128, 192], f32, kind="ExternalOutput").ap()
        DBG_H0BF = nc.dram_tensor("dbg_h0bf", [128, 80], f32, kind="ExternalOutput").ap()
        DBG_PG0 = nc.dram_tensor("dbg_pg0", [128, 512], f32, kind="ExternalOutput").ap()
        DBG_W = nc.dram_tensor("dbg_w", [128, 1024], f32, kind="ExternalOutput").ap()
        DBG_PG1 = nc.dram_tensor("dbg_pg1", [128, 512], f32, kind="ExternalOutput").ap()

    with tile.TileContext(nc) as tc:
        with ExitStack() as ctx:
            # persistent SBUF
            pers = ctx.enter_context(tc.tile_pool(name="pers", bufs=1))
            # rotating pools
            sb2 = ctx.enter_context(tc.tile_pool(name="sb2", bufs=2))
            sb3 = ctx.enter_context(tc.tile_pool(name="sb3", bufs=6))
            psg = ctx.enter_context(tc.tile_pool(name="psg", bufs=2, space="PSUM"))
            pst = ctx.enter_context(tc.tile_pool(name="pst", bufs=2, space="PSUM"))
            psx = ctx.enter_context(tc.tile_pool(name="psx", bufs=2, space="PSUM"))
            psp = ctx.enter_context(tc.tile_pool(name="psp", bufs=2, space="PSUM"))

            # ---------- persistent tiles ----------
            whhT = pers.tile([128, 4 * G3], bf16)
            wihT = pers.tile([128, 4 * G3], bf16)
            predwT = pers.tile([128, 4 * VS], bf16)
            predb = pers.tile([128, VS], bf16)
            bias_rz = pers.tile([128, 1024], bf16)
            bias_nT = pers.tile([128, 4], f32)
            bnb = pers.tile([32, 512], bf16)
            sel = pers.tile([128, 64], bf16)
            ident = pers.tile([128, 128], f32)
            ident_bf = pers.tile([128, 128], bf16)
            gx_rz = pers.tile([128, NT * 1024], bf16)   # [z|r] per tile
            gxnT = pers.tile([128, 4 * 2048], bf16)     # H-layout n-gate gx
            outT = pers.tile([128, 4 * 2048 + 16], bf16)  # H-layout hidden states (pred+rec lhsT)
            h0bf = pers.tile([128, 80], bf16)           # bf16 h0 for step-0 stationary
            state = pers.tile([128, 2 * 64], f32)        # rolling h (transposed), 2 slots
            gbuf = pers.tile([128, 512], bf16)          # B-layout gate staging window (bf16)

            nc.sync.dma_start(whhT[:], WHHT)
            nc.sync.dma_start(wihT[:], WIHT)
            nc.sync.dma_start(predwT[:], PREDWT)
            nc.sync.dma_start(predb[:], PREDB)
            nc.sync.dma_start(bias_rz[:], BIASRZ)
            nc.sync.dma_start(bias_nT[:], BIASNT)
            nc.sync.dma_start(bnb[:], BNB)
            nc.sync.dma_start(sel[:], SEL)
            make_identity(nc, ident[:])
            make_identity(nc, ident_bf[:])
            nc.gpsimd.memset(gbuf[:], 0.0)
            nc.gpsimd.memset(outT[:], 0.0)
            nc.gpsimd.memset(h0bf[:], 0.0)

            

            # ---------- fc -> h0 (into state slot 1, i.e. "h_{-1}") ----------
            hnT = sb2.tile([128, 8 * 16], f32r, tag="hnT")
            fcwT = pers.tile([128, 8 * 512], f32r)
            fcbT = sb2.tile([128, 4], f32, tag="fcbT")
            nc.sync.dma_start(hnT[:], HNT)
            nc.sync.dma_start(fcwT[:], FCWT)
            nc.sync.dma_start(fcbT[:], FCBT)
            pfc = psp.tile([128, 512], f32, tag="ppred")
            for kc in range(8):
                nc.tensor.matmul(
                    out=pfc[0:16, :],
                    lhsT=hnT[:, 16 * kc:16 * kc + 16],
                    rhs=fcwT[:, 512 * kc:512 * kc + 512],
                    start=(kc == 0), stop=(kc == 7),
                )
            fcbuf = sb2.tile([128, 512], f32, tag="fcbuf")
            nc.vector.tensor_copy(fcbuf[0:16, :], pfc[0:16, :])
            p2fc = pst.tile([128, 320], f32, tag="p2")
            for hc in range(4):
                nc.tensor.transpose(
                    out=p2fc[:, 16 * hc:16 * hc + 16],
                    in_=fcbuf[0:16, 128 * hc:128 * hc + 128],
                    identity=ident[0:16, 0:16],
                )
            for hc in range(4):
                nc.vector.tensor_scalar(
                    out=state[:, 64 + 16 * hc:64 + 16 * hc + 16],
                    in0=p2fc[:, 16 * hc:16 * hc + 16],
                    scalar1=fcbT[:, hc:hc + 1], scalar2=None, op0=ADD,
                )
            nc.vector.tensor_copy(h0bf[:, 0:64].rearrange("p (hc c) -> p hc c", hc=4),
                                  state[:, 64:128].rearrange("p (hc c) -> p hc c", hc=4))

            # ---------- helpers ----------

            def emit_gx_tile(j):
                ytile = sb3.tile([128, 1], i32, tag="ytile")
                nc.sync.dma_start(ytile[:], YT[128 * j:128 * j + 128, :])
                xt = sb3.tile([128, 512], f32, tag="xt")
                nc.gpsimd.indirect_dma_start(
                    out=xt[:], out_offset=None, in_=EMB,
                    in_offset=bass.IndirectOffsetOnAxis(ap=ytile[:, :1], axis=0),
                )
                px = psx.tile([128, 512], f32, tag="pgx")
                for ec in range(4):
                    nc.tensor.transpose(
                        out=px[:, 128 * ec:128 * ec + 128],
                        in_=xt[:, 128 * ec:128 * ec + 128],
                        identity=ident[:, :],
                    )
                xT = sb3.tile([128, 512], bf16, tag="xT")
                nc.vector.tensor_copy(xT[:], px[:])
                for g in range(3):
                    pgx = psx.tile([128, 512], f32, tag="pgx")
                    for ec in range(4):
                        nc.tensor.matmul(
                            out=pgx[:, :],
                            lhsT=xT[:, 128 * ec:128 * ec + 128],
                            rhs=wihT[:, 1536 * ec + 512 * g:1536 * ec + 512 * g + 512],
                            start=(ec == 0), stop=(ec == 3),
                        )
                    if g < 2:
                        nc.vector.tensor_tensor(
                            out=gx_rz[:, 1024 * j + 512 * g:1024 * j + 512 * g + 512],
                            in0=pgx[:, :], in1=bias_rz[:, 512 * g:512 * g + 512],
                            op=ADD,
                        )
                    else:
                        nstage = sb3.tile([128, 512], bf16, tag="nstage")
                        nc.vector.tensor_copy(nstage[:], pgx[:])
                        p2n = pst.tile([128, 512], bf16, tag="p2")
                        for hc in range(4):
                            nc.tensor.transpose(
                                out=p2n[:, 128 * hc:128 * hc + 128],
                                in_=nstage[:, 128 * hc:128 * hc + 128],
                                identity=ident_bf[:, :],
                            )
                        for hc in range(4):
                            nc.vector.tensor_scalar(
                                out=gxnT[:, 2048 * hc + 128 * j:2048 * hc + 128 * j + 128],
                                in0=p2n[:, 128 * hc:128 * hc + 128],
                                scalar1=bias_nT[:, hc:hc + 1], scalar2=None, op0=ADD,
                            )

            def emit_pred_tile(j):
                for v in range(8):
                    pp = psp.tile([128, 512], f32, tag="ppred")
                    nc.vector.tensor_copy(pp[:, 0:500], predb[:, 500 * v:500 * v + 500])
                    for k in range(4):
                        nc.tensor.matmul(
                            out=pp[:, 0:500],
                            lhsT=outT[:, 2048 * k + 128 * j:2048 * k + 128 * j + 128],
                            rhs=predwT[:, VS * k + 500 * v:VS * k + 500 * v + 500],
                            start=False, stop=(k == 3), skip_group_check=True,
                        )
                    ostage = sb3.tile([128, 500], f32, tag="ostage")
                    nc.scalar.copy(ostage[:, :], pp[:, 0:500])
                    nc.sync.dma_start(
                        OUT[128 * j:128 * j + 128, 500 * v:500 * v + 500],
                        ostage[:, :],
                    )

            def emit_step(t):
                jj = t % 8
                w = jj // 2            # 32-row window within gx tile
                par = jj % 2           # even/odd 16-row half
                tj = t // 8
                cur = t % 2            # state slot being written
                prv = 1 - cur
                sel_h = sel[32 * w:32 * w + 32, 32 * par:32 * par + 32]
                sel_h0 = sel[0:32, 32 * par:32 * par + 32]

                pg = psg.tile([128, 512], f32, tag="pg")
                # --- psum preloads (gx for z,r; bn for n) ---
                for grp in range(2):
                    nc.tensor.matmul(
                        out=pg[32 * grp:32 * grp + 32, :],
                        lhsT=sel_h,
                        rhs=gx_rz[32 * w:32 * w + 32,
                                  1024 * tj + 512 * grp:1024 * tj + 512 * grp + 512],
                        start=True, stop=False, skip_group_check=True,
                        tile_position=(32 * w, 32 * grp),
                    )
                nc.tensor.matmul(
                    out=pg[64:96, :], lhsT=sel_h0, rhs=bnb[:, :],
                    start=True, stop=False, skip_group_check=True,
                    tile_position=(0, 64),
                )
                if debug and t == 0:
                    nc.gpsimd.memset(dstage[:, 512:1024], 0.0)
                    for grp in range(3):
                        nc.vector.tensor_copy(dstage[32 * grp:32 * grp + 32, 512:1024],
                                              pg[32 * grp:32 * grp + 32, :])
                    nc.sync.dma_start(DBG_PG0, dstage[:, 512:1024])
                # --- recurrent matmuls, col-tiled across the 3 gate groups ---
                if t == 0:
                    st_bf = h0bf
                    sofs = 0
                else:
                    st_bf = outT
                    sofs = 16 * (t - 1)
                for k in range(4):
                    kofs = (2048 * k + sofs) if t > 0 else (16 * k)
                    for grp in range(3):
                        nc.tensor.matmul(
                            out=pg[32 * grp:32 * grp + 32, :],
                            lhsT=st_bf[:, kofs:kofs + 32],
                            rhs=whhT[:, 1536 * k + 512 * grp:1536 * k + 512 * grp + 512],
                            start=False, stop=(k == 3), skip_group_check=True,
                            tile_position=(0, 32 * grp),
                        )
                    if debug and t == 0 and k == 0:
                        for grp in range(3):
                            nc.vector.tensor_copy(
                                dstage[32 * grp:32 * grp + 32, 0:512],
                                pg[32 * grp:32 * grp + 32, :])
                        nc.sync.dma_start(DBG_PG1, dstage[:, 0:512])
                if debug and t == 0:
                    nc.gpsimd.memset(dstage[:, 0:512], 0.0)
                    for grp in range(3):
                        nc.vector.tensor_copy(dstage[32 * grp:32 * grp + 16, 0:512],
                                              pg[32 * grp:32 * grp + 16, :])
                    nc.sync.dma_start(DBG_PG, dstage[:, 0:512])
                # --- sigmoid z,r ; evict raw n ---
                nc.scalar.activation(gbuf[0:48, :], pg[0:48, :], SIG)
                nc.scalar.copy(gbuf[64:80, :], pg[64:80, :])
                # --- transpose combined window to H-layout ---
                p2 = pst.tile([128, 320], bf16, tag="p2")
                for hc in range(4):
                    nc.tensor.transpose(
                        out=p2[:, 80 * hc:80 * hc + 80],
                        in_=gbuf[0:80, 128 * hc:128 * hc + 128],
                        identity=ident_bf[0:80, 0:80],
                    )
                gateT = sb3.tile([128, 192], bf16, tag="gateT")
                p2v = p2[:, :].rearrange("p (hc c) -> p hc c", hc=4)
                p2src = bass.AP(p2v.tensor, p2v.offset,
                                [p2v.ap[0], p2v.ap[1], [32, 3], [1, 16]])
                gtv = gateT[:, :].rearrange("p (hc blk c) -> p hc blk c", hc=4, blk=3)
                nc.vector.tensor_copy(gtv, p2src)
                if debug and t == 0:
                    nc.sync.dma_start(DBG_GT, gateT[:, :])
                zT = gtv[:, :, 0, :]
                rT = gtv[:, :, 1, :]
                pnT = gtv[:, :, 2, :]
                # --- gate math in H-layout (FD=64) ---
                mb = sb3.tile([128, 64], bf16, tag="mb")
                nc.vector.tensor_tensor(
                    out=mb[:, :].rearrange("p (hc c) -> p hc c", hc=4),
                    in0=pnT, in1=rT, op=MUL)
                nin = sb3.tile([128, 64], bf16, tag="nin")
                gslice = gxnT[:, :].rearrange("p (hc c) -> p hc c", hc=4)[
                    :, :, 16 * t:16 * t + 16]
                nc.vector.tensor_tensor(
                    out=nin[:, :].rearrange("p (hc c) -> p hc c", hc=4),
                    in0=mb[:, :].rearrange("p (hc c) -> p hc c", hc=4),
                    in1=gslice, op=ADD)
                nT = sb3.tile([128, 64], f32, tag="nT")
                nc.scalar.activation(nT[:, :], nin[:, :], TANH)
                zp = sb3.tile([128, 64], f32, tag="zp")
                nc.vector.tensor_scalar(
                    out=zp[:, :].rearrange("p (hc c) -> p hc c", hc=4),
                    in0=zT, scalar1=-1.0, scalar2=1.0, op0=MUL, op1=ADD)
                zh = sb3.tile([128, 64], f32, tag="zh")
                nc.vector.tensor_tensor(
                    out=zh[:, :].rearrange("p (hc c) -> p hc c", hc=4), in0=zT,
                    in1=state[:, 64 * prv:64 * prv + 64].rearrange(
                        "p (hc c) -> p hc c", hc=4), op=MUL)
                t2 = sb3.tile([128, 64], f32, tag="t2")
                nc.vector.tensor_tensor(out=t2[:, :], in0=zp[:, :], in1=nT[:, :], op=MUL)
                st_new = state[:, 64 * cur:64 * cur + 64]
                nc.vector.tensor_tensor(out=st_new, in0=t2[:, :], in1=zh[:, :], op=ADD)
                # bf16 copy into outT (pred stationary), strided by h-chunk
                oslice = outT[:, 0:4 * 2048].rearrange("p (hc c) -> p hc c", hc=4)[
                    :, :, 16 * t:16 * t + 16]
                nc.vector.tensor_copy(
                    oslice, st_new.rearrange("p (hc c) -> p hc c", hc=4))

            if debug:
                dstage = pers.tile([128, 1024], f32)

            # ---------- schedule: gx lookahead 2 tiles, pred trails ----------
            if serial_gx:
                for j in range(NT):
                    emit_gx_tile(j)
            else:
                emit_gx_tile(0)
                emit_gx_tile(1)
            if debug:
                nc.vector.tensor_copy(dstage[:, 0:1024], whhT[:, 0:1024])
                nc.sync.dma_start(DBG_W, dstage[:, 0:1024])
                nc.vector.tensor_copy(dstage[:, 0:80], h0bf[:, :])
                nc.sync.dma_start(DBG_H0BF, dstage[:, 0:80])
                nc.vector.tensor_copy(dstage[:, 0:64], state[:, 64:128])
                nc.sync.dma_start(DBG_H0, dstage[:, 0:64])
                nc.vector.tensor_copy(dstage[:, 0:1024], gx_rz[:, 0:1024])
                nc.sync.dma_start(DBG_GX, dstage[:, 0:1024])
                gv = gxnT[:, 0:4 * 2048].rearrange("p (hc c) -> p hc c", hc=4)
                nc.vector.tensor_copy(
                    dstage[:, 0:256].rearrange("p (hc c) -> p hc c", hc=4),
                    gv[:, :, 0:64])
                nc.sync.dma_start(DBG_GXNT, dstage[:, 0:256])
            for j in range(NT):
                if j + 2 < NT and not serial_gx:
                    emit_gx_tile(j + 2)
                for t in range(8 * j, 8 * j + 8):
                    emit_step(t)
                if debug and j == 0:
                    ov = outT[:, 0:4 * 2048].rearrange("p (hc c) -> p hc c", hc=4)
                    nc.vector.tensor_copy(
                        dstage[:, 0:256].rearrange("p (hc c) -> p hc c", hc=4),
                        ov[:, :, 0:64])
                    nc.sync.dma_start(DBG_OUTT, dstage[:, 0:256])
                if not no_pred:
                    emit_pred_tile(j)

    nc.compile()
    _PROG_CACHE[key] = nc
    return nc


def prep_inputs(y, hn, emb, W_ih, W_hh, b_ih, b_hh, fc_w, fc_b, pred_w, pred_b):
    """Host-side layout prep. Returns per-core in_maps."""
    y = np.asarray(y)
    hn = np.asarray(hn, np.float32)
    emb = np.asarray(emb, np.float32)
    W_ih = np.asarray(W_ih, np.float32)
    W_hh = np.asarray(W_hh, np.float32)
    b_ih = np.asarray(b_ih, np.float32)
    b_hh = np.asarray(b_hh, np.float32)
    fc_w = np.asarray(fc_w, np.float32)
    fc_b = np.asarray(fc_b, np.float32)
    pred_w = np.asarray(pred_w, np.float32)
    pred_b = np.asarray(pred_b, np.float32)

    y_tm = np.ascontiguousarray(y.T.reshape(B * S, 1)).astype(np.int32)

    # hn [B,1,EH] -> hnT [128, 8*16]: hnT[p, kc*16+b] = hn[b,0,128kc+p]
    hn2 = hn[:, 0, :]                                  # [B, EH]
    hnT = np.zeros((128, 8 * 16), np.float32)
    for kc in range(8):
        hnT[:, 16 * kc:16 * kc + 16] = hn2[:, 128 * kc:128 * kc + 128].T
    # fcwT[p, kc*512+c] = fc_w[c, 128kc+p]
    fcwT = np.zeros((128, 8 * 512), np.float32)
    for kc in range(8):
        fcwT[:, 512 * kc:512 * kc + 512] = fc_w[:, 128 * kc:128 * kc + 128].T
    fcbT = np.ascontiguousarray(fc_b.reshape(4, 128).T)  # [128,4]

    # gate reorder: z, r, n  (reference order r,z,n)
    Wr, Wz, Wn = W_hh[:H], W_hh[H:2 * H], W_hh[2 * H:]
    Wg = np.concatenate([Wz, Wr, Wn], axis=0)          # [3H, H] in z,r,n order
    whhT = np.zeros((128, 4 * G3), np.float32)
    for k in range(4):
        whhT[:, G3 * k:G3 * k + G3] = Wg[:, 128 * k:128 * k + 128].T
    whhT = whhT.astype(ml_dtypes.bfloat16)
    WIr, WIz, WIn = W_ih[:H], W_ih[H:2 * H], W_ih[2 * H:]
    WIg = np.concatenate([WIz, WIr, WIn], axis=0)
    wihT = np.zeros((128, 4 * G3), np.float32)
    for k in range(4):
        wihT[:, G3 * k:G3 * k + G3] = WIg[:, 128 * k:128 * k + 128].T
    wihT = wihT.astype(ml_dtypes.bfloat16)

    bias_rz = np.zeros((128, 1024), np.float32)
    bias_rz[:, 0:512] = (b_ih[H:2 * H] + b_hh[H:2 * H])[None, :]   # z
    bias_rz[:, 512:1024] = (b_ih[0:H] + b_hh[0:H])[None, :]        # r
    bias_rz = bias_rz.astype(ml_dtypes.bfloat16)
    bias_nT = np.ascontiguousarray(b_ih[2 * H:].reshape(4, 128).T)  # [128,4] f32
    bnb = np.broadcast_to(b_hh[2 * H:][None, :], (32, 512)).astype(ml_dtypes.bfloat16)
    bnb = np.ascontiguousarray(bnb)

    selmat = np.zeros((32, 64), np.float32)
    for m in range(32):
        selmat[m % 32, m] = 1.0            # even: identity
        selmat[(16 + m) % 32, 32 + m] = 1.0  # odd: +16 rotation
    selmat = np.tile(selmat, (4, 1)).astype(ml_dtypes.bfloat16)  # [128, 64]

    in_maps = []
    for c in range(NC):
        pw = pred_w[VS * c:VS * c + VS]                # [VS, H]
        predwT = np.zeros((128, 4 * VS), np.float32)
        for k in range(4):
            predwT[:, VS * k:VS * k + VS] = pw[:, 128 * k:128 * k + 128].T
        predwT = predwT.astype(ml_dtypes.bfloat16)
        predb = np.broadcast_to(pred_b[VS * c:VS * c + VS][None, :], (128, VS))
        predb = np.ascontiguousarray(predb).astype(ml_dtypes.bfloat16)
        in_maps.append({
            "emb": emb, "y_tm": y_tm, "hnT": hnT, "fcwT": fcwT, "fcbT": fcbT,
            "whhT": whhT, "wihT": wihT, "bias_rz": bias_rz, "bias_nT": bias_nT,
            "bnb": bnb, "sel": selmat, "predwT": predwT, "predb": predb,
        })
    return in_maps


def kernel(**inputs):
    nc = build_program()
    in_maps = prep_inputs(**inputs)
    res = bass_utils.run_bass_kernel_spmd(nc, in_maps, core_ids=list(range(NC)))
    shards = [res.results[c]["out"].reshape(S, B, VS) for c in range(NC)]
    out = np.concatenate(shards, axis=-1)      # [S, B, V]
    return np.ascontiguousarray(out.transpose(1, 0, 2))  # [B, S, V]



# revision 38
# speedup vs baseline: 1.1632x; 1.1632x over previous
"""Trainium2 Bass kernel for nn_Decoder (GRU decoder + vocab projection).

Model (per reference):
    h0  = hn @ fc_w^T + fc_b                      [B,H]
    x   = emb[y]                                  [B,S,E]
    gx  = x @ W_ih^T + b_ih                       [B,S,3H]  (precomputed, biases folded)
    GRU scan over S steps (PyTorch gate order r,z,n):
        r = sigmoid(gxr + h@Wr^T + br_hh)
        z = sigmoid(gxz + h@Wz^T + bz_hh)
        n = tanh(gxn + r * (h@Wn^T + bn_hh))
        h = (1-z)*n + z*h
    out = h_seq @ pred_w^T + pred_b               [B,S,V]

Distribution: GRU scan replicated on all 8 cores; pred projection
vocab-sharded 8 ways; each core writes its [B*S, V/8] logit shard.

On-chip design (H-layout recurrence):
  - All recurrent state lives transposed: h^T as [H-dim partitions (4 chunks
    of 128), batch free].  The recurrent matmul runs with W_hh chunks as the
    stationary and h^T as the moving tensor, so each matmul streams only
    F=16 batch columns (48 small matmuls/step vs 12 big ones), and the state
    update lands directly in the layout the next step (and the pred matmul
    stationary) needs - no per-step transposes at all.
  - Gate psum bank [128, 192]: r chunks at cols 0:64, z at 64:128, n at
    128:192.  gx^T (+folded biases) preloaded into the r/z region and bn_hh
    into the n region by the Pool engine; PE matmuls accumulate on top.
  - sigmoid(r,z) on ACT straight from psum; n path: r*psum_n (+gxn) on DVE,
    tanh on ACT; update h' = (1-z)*n + z*h with (1-z), z*h computed on DVE
    in the tanh shadow.
  - gx^T computed tile-wise (128 (b,t) rows) with W_ih chunks stationary,
    emb gathered in bf16, transposed once per tile; biases folded via K=1
    bias-row matmuls; evicted to a per-step-interleaved SBUF layout
    [128, t*192 + 16m + b] so the per-step preload is one contiguous copy.
  - pred matmuls + gx matmuls interleave into PE idle slots during the scan
    (emission order: step MMs first, then gx slice, then pred slice).
  - all heavy matmuls in bf16 (1 cyc/row); fc in float32r.
"""
import numpy as np
import ml_dtypes
from contextlib import ExitStack

import concourse.bass as bass
import concourse.tile as tile
from concourse import bacc, mybir
from concourse import bass_utils
from concourse.masks import make_identity

f32 = mybir.dt.float32
f32r = mybir.dt.float32r
bf16 = mybir.dt.bfloat16
i32 = mybir.dt.int32
SIG = mybir.ActivationFunctionType.Sigmoid
TANH = mybir.ActivationFunctionType.Tanh
MUL = mybir.AluOpType.mult
ADD = mybir.AluOpType.add

V, E, EH, H = 32000, 512, 1024, 512
B, S = 16, 128
NC = 8
VS = V // NC          # 4000 vocab shard per core
NT = (B * S) // 128   # 16 (b,t)-tiles of 128 rows, t-major
G3 = 3 * H            # 1536
NM = 12               # gate-dim chunks of 128 (r:0-3, z:4-7, n:8-11)

_PROG_CACHE = {}


def build_program(debug=False):
    key = ("nc", debug)
    if key in _PROG_CACHE:
        return _PROG_CACHE[key]
    nc = bacc.Bacc("TRN2", target_bir_lowering=False, debug=False,
                   enable_asserts=False, num_devices=NC)

    # ---------------- DRAM I/O ----------------
    EMB = nc.dram_tensor("emb_bf", [V, E], bf16, kind="ExternalInput").ap()
    YSB = nc.dram_tensor("y_sb", [128, NT], i32, kind="ExternalInput").ap()
    HNT = nc.dram_tensor("hnT", [128, 8 * 16], bf16, kind="ExternalInput").ap()
    FCWT = nc.dram_tensor("fcwT", [128, 8 * 512], bf16, kind="ExternalInput").ap()
    FCBT = nc.dram_tensor("fcbT", [128, 4], f32, kind="ExternalInput").ap()
    WHHT = nc.dram_tensor("whhT", [128, 48 * 128], bf16, kind="ExternalInput").ap()
    WIHT = nc.dram_tensor("wihT", [128, 48 * 128], bf16, kind="ExternalInput").ap()
    BIASROW = nc.dram_tensor("biasrow", [1, G3], bf16, kind="ExternalInput").ap()
    ONES = nc.dram_tensor("ones_row", [1, 128], bf16, kind="ExternalInput").ap()
    BNBT = nc.dram_tensor("bnbT", [128, 64], bf16, kind="ExternalInput").ap()
    PREDWT = nc.dram_tensor("predwT", [128, 4 * VS], bf16, kind="ExternalInput").ap()
    PREDB = nc.dram_tensor("predb", [128, VS], bf16, kind="ExternalInput").ap()
    OUT = nc.dram_tensor("out", [B * S, VS], f32, kind="ExternalOutput").ap()
    if debug:
        DBG_GXT = nc.dram_tensor("dbg_gxt", [128, 1024], f32, kind="ExternalOutput").ap()
        DBG_ZR = nc.dram_tensor("dbg_zr", [128, 128], f32, kind="ExternalOutput").ap()
        DBG_ST0 = nc.dram_tensor("dbg_st0", [128, 64], f32, kind="ExternalOutput").ap()
        DBG_H0 = nc.dram_tensor("dbg_h0", [128, 64], f32, kind="ExternalOutput").ap()
        DBG_OUTT = nc.dram_tensor("dbg_outT", [128, 256], f32, kind="ExternalOutput").ap()
        DBG_PG = nc.dram_tensor("dbg_pg", [128, 192], f32, kind="ExternalOutput").ap()

    with tile.TileContext(nc) as tc:
        with ExitStack() as ctx:
            pers = ctx.enter_context(tc.tile_pool(name="pers", bufs=1))
            sb2 = ctx.enter_context(tc.tile_pool(name="sb2", bufs=2))
            sb3 = ctx.enter_context(tc.tile_pool(name="sb3", bufs=2))
            psg = ctx.enter_context(tc.tile_pool(name="psg", bufs=2, space="PSUM"))
            psgx = ctx.enter_context(tc.tile_pool(name="psgx", bufs=1, space="PSUM"))
            psx = ctx.enter_context(tc.tile_pool(name="psx", bufs=1, space="PSUM"))
            psp = ctx.enter_context(tc.tile_pool(name="psp", bufs=2, space="PSUM"))

            # ---------- persistent tiles ----------
            whhT = pers.tile([128, 48 * 128], bf16)
            wihT = pers.tile([128, 48 * 128], bf16)
            predwT = pers.tile([128, 4 * VS], bf16)
            predb = pers.tile([128, VS], bf16)
            biasrow = pers.tile([1, G3], bf16)
            onesr = pers.tile([1, 128], bf16)
            bnbT = pers.tile([128, 64], bf16)
            ident32 = pers.tile([128, 128], f32)
            ident_bf = pers.tile([128, 128], bf16)
            y_sb = pers.tile([128, NT], i32)
            gxT = pers.tile([128, S * 192], bf16)       # per-step interleaved gx^T
            outT = pers.tile([128, 4 * 2048], bf16)     # h^T history (pred lhsT + rec rhs)
            h0bf = pers.tile([128, 64], bf16)
            state = pers.tile([128, 2 * 64], f32)       # fc staging for h0
            fcwT = pers.tile([128, 8 * 512], bf16)

            # DMA order tuned for the prologue critical path: fc/gather
            # inputs first, then the matmul weights in the order the PE
            # needs them; bias tiles (only needed at the gx stop-MMs /
            # step preloads) slot in before the pred weights.
            hnT = sb2.tile([128, 8 * 16], bf16, tag="hnT")
            fcbT = sb2.tile([128, 4], f32, tag="fcbT")
            nc.sync.dma_start(y_sb[:], YSB)
            nc.sync.dma_start(hnT[:], HNT)
            nc.sync.dma_start(fcbT[:], FCBT)
            nc.sync.dma_start(biasrow[:], BIASROW)
            nc.sync.dma_start(onesr[:], ONES)
            nc.sync.dma_start(bnbT[:], BNBT)
            nc.sync.dma_start(fcwT[:], FCWT)
            nc.sync.dma_start(wihT[:], WIHT)
            nc.sync.dma_start(whhT[:], WHHT)
            nc.sync.dma_start(predb[:], PREDB)
            nc.sync.dma_start(predwT[:], PREDWT)
            make_identity(nc, ident32[:])
            make_identity(nc, ident_bf[:])

            # PE p-state warmup: dummy identity matmuls keep the PE busy
            # while the weight DMAs stream in, so the real matmuls start at
            # full clock instead of paying the 3us ramp.
            warm = psg.tile([128, 192], f32, tag="pg")
            for i in range(28):
                nc.tensor.matmul(
                    out=warm[:, 0:128], lhsT=ident_bf[:, :], rhs=ident_bf[:, :],
                    start=(i == 0), stop=(i == 27), skip_group_check=True,
                )

            # ---------- fc -> h0 (emitted as a callable; scheduled after
            # gx tile 0's matmuls so the PE p-state is warm) ----------
            def emit_fc():
                pfc = psp.tile([128, 512], f32, tag="pp")
                for kc in range(8):
                    nc.tensor.matmul(
                        out=pfc[0:16, :],
                        lhsT=hnT[:, 16 * kc:16 * kc + 16],
                        rhs=fcwT[:, 512 * kc:512 * kc + 512],
                        start=(kc == 0), stop=(kc == 7),
                    )
                fcbuf = sb2.tile([128, 512], f32, tag="fcbuf")
                nc.vector.tensor_copy(fcbuf[0:16, :], pfc[0:16, :])
                p2fc = psg.tile([128, 192], f32, tag="pg")
                for hc in range(4):
                    nc.tensor.transpose(
                        out=p2fc[:, 16 * hc:16 * hc + 16],
                        in_=fcbuf[0:16, 128 * hc:128 * hc + 128],
                        identity=ident32[0:16, 0:16],
                    )
                for hc in range(4):
                    nc.vector.tensor_scalar(
                        out=state[:, 64 + 16 * hc:64 + 16 * hc + 16],
                        in0=p2fc[:, 16 * hc:16 * hc + 16],
                        scalar1=fcbT[:, hc:hc + 1], scalar2=None, op0=ADD,
                    )
                nc.vector.tensor_copy(h0bf[:], state[:, 64:128])

            # ---------- gx tile pipeline (pieces indexed s=0..7) ----------
            gx_ctx = {}   # jj -> dict(xt=, xT=, gxp=)

            def emit_gx_piece(jj, s):
                if s == 0:
                    xt = sb3.tile([128, 512], bf16, tag="xt", bufs=3)
                    nc.gpsimd.indirect_dma_start(
                        out=xt[:], out_offset=None, in_=EMB,
                        in_offset=bass.IndirectOffsetOnAxis(
                            ap=y_sb[:, jj:jj + 1], axis=0),
                    )
                    gx_ctx[jj] = {"xt": xt}
                elif s == 1:
                    c = gx_ctx[jj]
                    px = psx.tile([128, 512], bf16, tag="px")
                    for ec in range(4):
                        nc.tensor.transpose(
                            out=px[:, 128 * ec:128 * ec + 128],
                            in_=c["xt"][:, 128 * ec:128 * ec + 128],
                            identity=ident_bf[:, :],
                        )
                    xT = sb3.tile([128, 512], bf16, tag="xT")
                    nc.vector.tensor_copy(xT[:], px[:])
                    c["xT"] = xT
                    gxp = psgx.tile([128, NM * 128], f32, tag="gxp")
                    c["gxp"] = gxp
                else:
                    c = gx_ctx[jj]
                    gxp, xT = c["gxp"], c["xT"]
                    for m in (2 * (s - 2), 2 * (s - 2) + 1):
                        for k in range(4):
                            nc.tensor.matmul(
                                out=gxp[:, 128 * m:128 * m + 128],
                                lhsT=wihT[:, (k * NM + m) * 128:(k * NM + m) * 128 + 128],
                                rhs=xT[:, 128 * k:128 * k + 128],
                                start=(k == 0), stop=False, skip_group_check=True,
                            )
                        nc.tensor.matmul(
                            out=gxp[:, 128 * m:128 * m + 128],
                            lhsT=biasrow[0:1, 128 * m:128 * m + 128],
                            rhs=onesr[0:1, :],
                            start=False, stop=True, skip_group_check=True,
                        )
                    if s in (3, 5, 7):
                        h = (s - 3) // 2
                        sv = gxp[:, 512 * h:512 * h + 512]
                        src = bass.AP(sv.tensor, sv.offset,
                                      [sv.ap[0], [128, 4], [16, 8], [1, 16]])
                        dv = gxT[:, 1536 * jj + 64 * h:1536 * jj + 64 * h + 1]
                        dst = bass.AP(dv.tensor, dv.offset,
                                      [dv.ap[0], [16, 4], [192, 8], [1, 16]])
                        nc.scalar.copy(dst, src)
                        if jj in gx_ctx and s == 7:
                            del gx_ctx[jj]

            # ---------- one GRU step ----------
            def emit_step(t):
                pg = psg.tile([128, 192], f32, tag="pg")
                # psum preload via identity matmuls: gx^T(+biases) into the
                # r/z region, bn_hh into the n region.  start=True marks the
                # whole bank pending-zero; the second preload and the k=0
                # accumulates then overwrite their (still-pending) bytes, and
                # k>0 accumulates add on top.
                nc.tensor.matmul(
                    out=pg[:, 0:128], lhsT=ident_bf[:, :],
                    rhs=gxT[:, 192 * t:192 * t + 128],
                    start=True, stop=False, skip_group_check=True,
                )
                nc.tensor.matmul(
                    out=pg[:, 128:192], lhsT=ident_bf[:, :], rhs=bnbT[:],
                    start=False, stop=False, skip_group_check=True,
                )
                # previous h^T (bf16): h0 staging tile for t=0, else outT cols
                if t == 0:
                    hprev = h0bf[:]
                    hprev4 = h0bf[:].rearrange("p (k c) -> p k c", k=4)
                else:
                    hprev4 = outT[:, :].rearrange("p (k c) -> p k c", k=4)[
                        :, :, 16 * (t - 1):16 * (t - 1) + 16]
                    hprev = None
                # recurrent matmuls: W_hh chunks stationary, h^T moving (F=16)
                for m in range(NM):
                    for k in range(4):
                        if t == 0:
                            rhs = h0bf[:, 16 * k:16 * k + 16]
                        else:
                            rhs = outT[:, 2048 * k + 16 * (t - 1):2048 * k + 16 * (t - 1) + 16]
                        nc.tensor.matmul(
                            out=pg[:, 16 * m:16 * m + 16],
                            lhsT=whhT[:, (k * NM + m) * 128:(k * NM + m) * 128 + 128],
                            rhs=rhs,
                            start=False, stop=(k == 3), skip_group_check=True,
                        )
                if debug and t == 0:
                    dpg = pers.tile([128, 192], f32)
                    nc.vector.tensor_copy(dpg[:], pg[:, 0:192])
                    nc.sync.dma_start(DBG_PG, dpg[:])
                # sigma(r) first and the n path right behind it — sigma(z) is
                # emitted AFTER rn/nin so their sem waits don't cover it
                r_sb = sb3.tile([128, 64], bf16, tag="r_sb")
                z_sb = sb3.tile([128, 64], bf16, tag="z_sb")
                nc.scalar.activation(r_sb[:], pg[:, 0:64], SIG)
                rn = sb3.tile([128, 64], bf16, tag="rn")
                nc.vector.tensor_tensor(out=rn[:], in0=r_sb[:],
                                        in1=pg[:, 128:192], op=MUL)
                nin = sb3.tile([128, 64], bf16, tag="nin")
                nc.vector.tensor_tensor(out=nin[:], in0=rn[:],
                                        in1=gxT[:, 192 * t + 128:192 * t + 192], op=ADD)
                nc.scalar.activation(z_sb[:], pg[:, 64:128], SIG)
                if debug and t == 0:
                    dzr = pers.tile([128, 128], f32)
                    nc.vector.tensor_copy(dzr[:, 0:64], r_sb[:])
                    nc.vector.tensor_copy(dzr[:, 64:128], z_sb[:])
                    nc.sync.dma_start(DBG_ZR, dzr[:])
                nT = sb3.tile([128, 64], bf16, tag="nT")
                nc.scalar.activation(nT[:], nin[:], TANH)
                # z terms (overlap tanh), all bf16 for DVE fast modes
                zp = sb3.tile([128, 64], bf16, tag="zp")
                nc.vector.tensor_scalar(out=zp[:], in0=z_sb[:],
                                        scalar1=-1.0, scalar2=1.0, op0=MUL, op1=ADD)
                zh = sb3.tile([128, 64], bf16, tag="zh")
                if t == 0:
                    nc.vector.tensor_tensor(out=zh[:], in0=z_sb[:],
                                            in1=hprev, op=MUL)
                else:
                    nc.vector.tensor_tensor(
                        out=zh[:].rearrange("p (k c) -> p k c", k=4),
                        in0=z_sb[:].rearrange("p (k c) -> p k c", k=4),
                        in1=hprev4, op=MUL)
                # h' = (1-z)*n + z*h
                t2 = sb3.tile([128, 64], bf16, tag="t2")
                nc.vector.tensor_tensor(out=t2[:], in0=zp[:], in1=nT[:], op=MUL)
                oslice = outT[:, :].rearrange("p (k c) -> p k c", k=4)[
                    :, :, 16 * t:16 * t + 16]
                nc.vector.tensor_tensor(
                    out=oslice,
                    in0=t2[:].rearrange("p (k c) -> p k c", k=4),
                    in1=zh[:].rearrange("p (k c) -> p k c", k=4), op=ADD)

            # ---------- pred slice (tile jj, vocab column v) ----------
            def emit_pred_slice(jj, v):
                pp = psp.tile([128, 500], f32, tag="pp")
                for k in range(4):
                    nc.tensor.matmul(
                        out=pp[:, :],
                        lhsT=outT[:, 2048 * k + 128 * jj:2048 * k + 128 * jj + 128],
                        rhs=predwT[:, VS * k + 500 * v:VS * k + 500 * v + 500],
                        start=(k == 0), stop=(k == 3), skip_group_check=True,
                    )
                # bias folded into the eviction copy
                # GPSIMD cannot touch PSUM on real HW; bias-add eviction on DVE
                ostage = sb3.tile([128, 500], f32, tag="ostage")
                nc.vector.tensor_tensor(out=ostage[:, :], in0=pp[:, :],
                                        in1=predb[:, 500 * v:500 * v + 500], op=ADD)
                nc.sync.dma_start(
                    OUT[128 * jj:128 * jj + 128, 500 * v:500 * v + 500],
                    ostage[:, :],
                )

            # ---------- schedule ----------
            emit_fc()
            for s in range(8):
                emit_gx_piece(0, s)
            for s in range(8):
                emit_gx_piece(1, s)
            for j in range(NT):
                for s in range(8):
                    t = 8 * j + s
                    emit_step(t)
                    if j + 2 < NT:
                        emit_gx_piece(j + 2, s)
                    if j >= 1:
                        emit_pred_slice(j - 1, s)
                    if debug and t == 0:
                        dstage = pers.tile([128, 1024], f32)
                        nc.vector.tensor_copy(dstage[:, 0:1024], gxT[:, 0:1024])
                        nc.sync.dma_start(DBG_GXT, dstage[:, 0:1024])
                        dst0 = pers.tile([128, 64], f32)
                        nc.vector.tensor_copy(
                            dst0[:].rearrange("p (k c) -> p k c", k=4),
                            outT[:, :].rearrange("p (k c) -> p k c", k=4)[:, :, 0:16])
                        nc.sync.dma_start(DBG_ST0, dst0[:])
                        nc.sync.dma_start(DBG_H0, state[:, 64:128])
            for s in range(8):
                emit_pred_slice(NT - 1, s)
            if debug:
                dst2 = pers.tile([128, 256], f32)
                nc.vector.tensor_copy(
                    dst2[:, :].rearrange("p (k c) -> p k c", k=4),
                    outT[:, :].rearrange("p (k c) -> p k c", k=4)[:, :, 0:64])
                nc.sync.dma_start(DBG_OUTT, dst2[:, :])

    nc.compile()
    _PROG_CACHE[key] = nc
    return nc


def prep_inputs(y, hn, emb, W_ih, W_hh, b_ih, b_hh, fc_w, fc_b, pred_w, pred_b):
    """Host-side layout prep. Returns per-core in_maps."""
    y = np.asarray(y)
    hn = np.asarray(hn, np.float32)
    emb = np.asarray(emb, np.float32)
    W_ih = np.asarray(W_ih, np.float32)
    W_hh = np.asarray(W_hh, np.float32)
    b_ih = np.asarray(b_ih, np.float32)
    b_hh = np.asarray(b_hh, np.float32)
    fc_w = np.asarray(fc_w, np.float32)
    fc_b = np.asarray(fc_b, np.float32)
    pred_w = np.asarray(pred_w, np.float32)
    pred_b = np.asarray(pred_b, np.float32)

    # y in t-major layout packed [128, NT]: col j holds rows 128j..128j+127
    y_tm = y.T.reshape(B * S).astype(np.int32)        # row = 16t + b
    y_sb = np.ascontiguousarray(y_tm.reshape(NT, 128).T)

    emb_bf = emb.astype(ml_dtypes.bfloat16)

    # hn [B,1,EH] -> hnT [128, 8*16]: hnT[p, kc*16+b] = hn[b,0,128kc+p]
    hn2 = hn[:, 0, :]
    hnT = np.zeros((128, 8 * 16), np.float32)
    for kc in range(8):
        hnT[:, 16 * kc:16 * kc + 16] = hn2[:, 128 * kc:128 * kc + 128].T
    hnT = hnT.astype(ml_dtypes.bfloat16)
    fcwT = np.zeros((128, 8 * 512), np.float32)
    for kc in range(8):
        fcwT[:, 512 * kc:512 * kc + 512] = fc_w[:, 128 * kc:128 * kc + 128].T
    fcwT = fcwT.astype(ml_dtypes.bfloat16)
    fcbT = np.ascontiguousarray(fc_b.reshape(4, 128).T)  # [128,4]

    # gate order r,z,n = reference W_hh order; m-chunks of 128 gate dims
    whhT = np.zeros((128, 48 * 128), np.float32)
    wihT = np.zeros((128, 48 * 128), np.float32)
    for k in range(4):
        for m in range(NM):
            blk = (k * NM + m) * 128
            whhT[:, blk:blk + 128] = W_hh[128 * m:128 * m + 128,
                                          128 * k:128 * k + 128].T
            wihT[:, blk:blk + 128] = W_ih[128 * m:128 * m + 128,
                                          128 * k:128 * k + 128].T
    whhT = whhT.astype(ml_dtypes.bfloat16)
    wihT = wihT.astype(ml_dtypes.bfloat16)

    # bias folded into gx: r,z get b_ih+b_hh; n gets b_ih only
    biasrow = np.concatenate([
        b_ih[0:2 * H] + b_hh[0:2 * H], b_ih[2 * H:]
    ]).reshape(1, G3).astype(ml_dtypes.bfloat16)
    ones_row = np.ones((1, 128), np.float32).astype(ml_dtypes.bfloat16)
    # bn_hh broadcast tile [128, 64]: col 16*nm+b = b_hh[2H + 128nm + p]
    bnbT = np.zeros((128, 64), np.float32)
    for nm in range(4):
        bnbT[:, 16 * nm:16 * nm + 16] = b_hh[2 * H + 128 * nm:
                                             2 * H + 128 * nm + 128][:, None]
    bnbT = bnbT.astype(ml_dtypes.bfloat16)

    in_maps = []
    for c in range(NC):
        pw = pred_w[VS * c:VS * c + VS]                # [VS, H]
        predwT = np.zeros((128, 4 * VS), np.float32)
        for k in range(4):
            predwT[:, VS * k:VS * k + VS] = pw[:, 128 * k:128 * k + 128].T
        predwT = predwT.astype(ml_dtypes.bfloat16)
        predb = np.broadcast_to(pred_b[VS * c:VS * c + VS][None, :], (128, VS))
        predb = np.ascontiguousarray(predb).astype(ml_dtypes.bfloat16)
        in_maps.append({
            "emb_bf": emb_bf, "y_sb": y_sb, "hnT": hnT, "fcwT": fcwT,
            "fcbT": fcbT, "whhT": whhT, "wihT": wihT, "biasrow": biasrow,
            "ones_row": ones_row, "bnbT": bnbT, "predwT": predwT,
            "predb": predb,
        })
    return in_maps


def kernel(**inputs):
    nc = build_program()
    in_maps = prep_inputs(**inputs)
    res = bass_utils.run_bass_kernel_spmd(nc, in_maps, core_ids=list(range(NC)))
    shards = [res.results[c]["out"].reshape(S, B, VS) for c in range(NC)]
    out = np.concatenate(shards, axis=-1)      # [S, B, V]
    return np.ascontiguousarray(out.transpose(1, 0, 2))  # [B, S, V]
